# revision 6
# baseline (speedup 1.0000x reference)
"""Trainium2 Bass kernel: decoder self-attention with KV cache (AttentionForONNX).

Data-parallel over batch: B=16 batches sharded 2-per-core across 8 NeuronCores.
Each core computes q/k/v projections, attention over its 2 batches (16 heads,
L=2048 cache + S=128 new positions), and the output projection. No collectives.

Layout strategy (all matmuls natural, zero on-device transposes):
  - scores computed TRANSPOSED [t, s]: lhsT = kT chunk [hd=64, t=128]
    (even head at partitions 0-63, odd at 64-127 -> concurrent row-tiled MMs),
    rhs = qT head slice [hd=64, s=128].
  - softmax denominator via ones-column appended to V (extra PSUM row),
    no max-subtraction (scores are bounded ~|4|, exp is safe).
  - AV: lhsT = v_aug chunk [t=128, 65], rhs = expT chunk [t=128, s=128];
    output [65, s] accumulates over 17 chunks; row 64 = denominator.
  - divide: reciprocal rows -> one K=1 PE broadcast matmul -> bulk DVE multiply.
  - out projection: lhsT = YT chunk (stationary), rhs = WoT.
"""

import numpy as np
import ml_dtypes

import concourse.bass as bass
import concourse.mybir as mybir
from concourse import bacc
from concourse.tile import TileContext
from concourse.bass_utils import run_bass_kernel_spmd

AF = mybir.ActivationFunctionType
ALU = mybir.AluOpType
BF16 = mybir.dt.bfloat16
F32 = mybir.dt.float32
bf16np = ml_dtypes.bfloat16

S, B, E = 128, 16, 1024
NH, HD, L = 16, 64, 2048
NCORES = 8
BL = B // NCORES          # 2 batches per core
R = S * BL                # 256 rows (s-major, b-minor)
T = L + S                 # 2176 total kv positions
NTC = L // 128            # 16 cache chunks (+1 new chunk)
NKO = E // 128            # 8 e-blocks
SCALE = HD ** -0.5


def build_graph():
    nc = bacc.Bacc(target_bir_lowering=False)

    xt = nc.declare_dram_parameter("xt", [128, NKO, S, BL], BF16, isOutput=False)
    kt = nc.declare_dram_parameter("kt", [BL, NH // 2, 128, L], BF16, isOutput=False)
    vt = nc.declare_dram_parameter("vt", [BL, NH, 128, NTC, HD], BF16, isOutput=False)
    wq = nc.declare_dram_parameter("wq", [128, NKO, E], BF16, isOutput=False)
    wk = nc.declare_dram_parameter("wk", [128, NKO, E], BF16, isOutput=False)
    wv = nc.declare_dram_parameter("wv", [128, NKO, E], BF16, isOutput=False)
    wo = nc.declare_dram_parameter("wo", [128, NKO, E], BF16, isOutput=False)
    bq = nc.declare_dram_parameter("bq", [128, NKO], F32, isOutput=False)
    bk = nc.declare_dram_parameter("bk", [128, NKO], F32, isOutput=False)
    bvb = nc.declare_dram_parameter("bvb", [128, E], F32, isOutput=False)
    bob = nc.declare_dram_parameter("bob", [128, E], F32, isOutput=False)

    out = nc.declare_dram_parameter("out", [128, BL, E], F32, isOutput=True)
    knewt = nc.declare_dram_parameter("knewt", [128, NKO, S, BL], F32, isOutput=True)
    vnew = nc.declare_dram_parameter("vnew", [BL, 128, E], F32, isOutput=True)

    with TileContext(nc) as tc:
        with (
            tc.tile_pool(name="const", bufs=1) as cpool,
            tc.tile_pool(name="wpool", bufs=1) as wpool,
            tc.tile_pool(name="acts", bufs=1) as apool,
        ):
            # ---- persistent loads ----
            xt_sb = apool.tile([128, NKO, S, BL], BF16)
            nc.sync.dma_start(xt_sb[:], xt[:])
            wq_sb = wpool.tile([128, NKO, E], BF16, tag="wq")
            nc.sync.dma_start(wq_sb[:], wq[:])
            wk_sb = wpool.tile([128, NKO, E], BF16, tag="wk")
            nc.sync.dma_start(wk_sb[:], wk[:])
            wv_sb = wpool.tile([128, NKO, E], BF16, tag="wv")
            nc.sync.dma_start(wv_sb[:], wv[:])
            wo_sb = wpool.tile([128, NKO, E], BF16, tag="wo")
            nc.sync.dma_start(wo_sb[:], wo[:])
            bq_sb = cpool.tile([128, NKO], F32, tag="bq")
            nc.sync.dma_start(bq_sb[:], bq[:])
            bk_sb = cpool.tile([128, NKO], F32, tag="bk")
            nc.sync.dma_start(bk_sb[:], bk[:])
            bvb_sb = cpool.tile([128, E], F32, tag="bvb")
            nc.sync.dma_start(bvb_sb[:], bvb[:])
            bob_sb = cpool.tile([128, E], F32, tag="bob")
            nc.sync.dma_start(bob_sb[:], bob[:])
            ones_col = cpool.tile([1, 128], F32, tag="ones")
            nc.vector.memset(ones_col[:], 1.0)

            qt_bf = apool.tile([128, NKO, S, BL], BF16)
            knew_bf = apool.tile([128, NKO, S, BL], BF16)
            knew_f32 = apool.tile([128, NKO, S, BL], F32)
            vnew_f32 = [apool.tile([128, E], F32, tag=f"vnf{b}", name=f"vnf{b}") for b in range(BL)]
            vnew_aug = [apool.tile([128, NH, HD + 1], BF16, tag=f"vna{b}", name=f"vna{b}") for b in range(BL)]
            yt_f32 = apool.tile([128, NKO, S, BL], F32)
            yt_bf = apool.tile([128, NKO, S, BL], BF16)
            # reciprocal denominators, laid out [ko, half, s, b]
            drec = apool.tile([1, NKO, 2, S, BL], F32)

            # ---- phase 1: projections ----
            with tc.tile_pool(name="pproj", bufs=2, space="PSUM") as pp:
                for mo in range(NKO):
                    ps = pp.tile([128, R], F32, tag="pqk")
                    for ki in range(NKO):
                        nc.tensor.matmul(
                            ps[:],
                            wq_sb[:, ki, mo * 128:(mo + 1) * 128],
                            xt_sb[:, ki].rearrange("p s b -> p (s b)"),
                            start=(ki == 0), stop=(ki == NKO - 1),
                        )
                    nc.scalar.activation(
                        qt_bf[:, mo].rearrange("p s b -> p (s b)"), ps[:], AF.Identity,
                        bias=bq_sb[:, mo:mo + 1], scale=1.0,
                    )
                for mo in range(NKO):
                    ps = pp.tile([128, R], F32, tag="pqk")
                    for ki in range(NKO):
                        nc.tensor.matmul(
                            ps[:],
                            wk_sb[:, ki, mo * 128:(mo + 1) * 128],
                            xt_sb[:, ki].rearrange("p s b -> p (s b)"),
                            start=(ki == 0), stop=(ki == NKO - 1),
                        )
                    nc.scalar.activation(
                        knew_bf[:, mo].rearrange("p s b -> p (s b)"), ps[:], AF.Identity,
                        bias=bk_sb[:, mo:mo + 1], scale=1.0,
                    )
                    nc.vector.tensor_scalar_add(
                        knew_f32[:, mo].rearrange("p s b -> p (s b)"), ps[:], bk_sb[:, mo:mo + 1]
                    )
                nc.sync.dma_start(knewt[:], knew_f32[:])

                # v_new natural per batch: out[s, eo]
                for b in range(BL):
                    for nhalf in range(2):
                        ps = pp.tile([128, 512], F32, tag="pv")
                        for ki in range(NKO):
                            nc.tensor.matmul(
                                ps[:],
                                xt_sb[:, ki, :, b],
                                wv_sb[:, ki, nhalf * 512:(nhalf + 1) * 512],
                                start=(ki == 0), stop=(ki == NKO - 1),
                            )
                        nc.vector.tensor_tensor(
                            vnew_f32[b][:, nhalf * 512:(nhalf + 1) * 512],
                            ps[:], bvb_sb[:, nhalf * 512:(nhalf + 1) * 512],
                            ALU.add,
                        )
                    nc.sync.dma_start(vnew[b], vnew_f32[b][:])
                    nc.vector.tensor_copy(
                        out=vnew_aug[b][:, :, 0:HD],
                        in_=vnew_f32[b][:].rearrange("p (h d) -> p h d", d=HD),
                    )
                    nc.vector.memset(vnew_aug[b][:, :, HD:HD + 1], 1.0)

            # ---- phase 2: attention ----
            # Per head: 17 scoresT chunk matmuls into two multi-bank PSUM
            # tiles (8 + 9 chunks), ONE batched exp per tile (ACT cost is
            # (N+352)/1.2ns -- big N amortizes the fixed cost), then 17 AV
            # matmuls accumulating [65, s] (row 64 = softmax denominator).
            with (
                tc.tile_pool(name="kp", bufs=3) as kpool,
                tc.tile_pool(name="vp", bufs=3) as vpool,
                tc.tile_pool(name="ep", bufs=2) as epool,
                tc.tile_pool(name="pssA", bufs=2, space="PSUM") as pssA,
                tc.tile_pool(name="pssB", bufs=1, space="PSUM") as pssB,
                tc.tile_pool(name="pso", bufs=1, space="PSUM") as pso,
            ):
                for b in range(BL):
                    for hp in range(NH // 2):
                        kslab = kpool.tile([128, L], BF16, tag="kslab")
                        nc.sync.dma_start(kslab[:], kt[b, hp])
                        vaug = []
                        for j in range(2):
                            va = vpool.tile([128, NTC, HD + 1], BF16, tag=f"vaug{j}", name=f"vaug{j}")
                            nc.sync.dma_start(va[:, :, 0:HD], vt[b, 2 * hp + j])
                            nc.vector.memset(va[:, :, HD:HD + 1], 1.0)
                            vaug.append(va)

                        for j in range(2):
                            h = 2 * hp + j
                            qh = qt_bf[64 * j:64 * (j + 1), hp, :, b]
                            tA = pssA.tile([128, 8, 128], F32, tag="sA")
                            tB = pssB.tile([128, 9, 128], F32, tag="sB")
                            po = pso.tile([65, 128], F32, tag="po")
                            et = epool.tile([128, NTC + 1, 128], BF16, tag="exp")
                            for tci in range(8):
                                lhs = kslab[64 * j:64 * (j + 1), tci * 128:(tci + 1) * 128]
                                nc.tensor.matmul(tA[:, tci, :], lhs, qh, start=True, stop=True)
                            nc.scalar.activation(et[:, 0:8, :], tA[:], AF.Exp)
                            for tci in range(8, NTC + 1):
                                if tci < NTC:
                                    lhs = kslab[64 * j:64 * (j + 1), tci * 128:(tci + 1) * 128]
                                else:
                                    lhs = knew_bf[64 * j:64 * (j + 1), hp, :, b]
                                nc.tensor.matmul(tB[:, tci - 8, :], lhs, qh, start=True, stop=True)
                            nc.scalar.activation(et[:, 8:NTC + 1, :], tB[:], AF.Exp)
                            for tci in range(NTC + 1):
                                if tci < NTC:
                                    lhsv = vaug[j][:, tci, :]
                                else:
                                    lhsv = vnew_aug[b][:, h, :]
                                nc.tensor.matmul(
                                    po[:], lhsv, et[:, tci, :],
                                    start=(tci == 0), stop=(tci == NTC),
                                )
                            nc.vector.reciprocal(drec[0:1, hp, j, :, b], po[64:65, :])
                            nc.vector.tensor_copy(
                                out=yt_f32[64 * j:64 * (j + 1), hp, :, b],
                                in_=po[0:64, :],
                            )

            # ---- phase 3: divide + out projection ----
            with (
                tc.tile_pool(name="pbc", bufs=2, space="PSUM") as pbc,
                tc.tile_pool(name="pout", bufs=2, space="PSUM") as pout,
                tc.tile_pool(name="osb", bufs=3) as opool,
            ):
                for ko in range(NKO):
                    rb = pbc.tile([128, 512], F32, tag="rb")
                    nc.tensor.matmul(
                        rb[:], ones_col[:], drec[0:1, ko].rearrange("p a s b -> p (a s b)"),
                        start=True, stop=True,
                    )
                    for j in range(2):
                        nc.vector.tensor_tensor(
                            yt_bf[64 * j:64 * (j + 1), ko, :, :],
                            yt_f32[64 * j:64 * (j + 1), ko, :, :],
                            rb[64 * j:64 * (j + 1), j * 256:(j + 1) * 256].rearrange("p (s b) -> p s b", b=BL),
                            ALU.mult,
                        )

                for b in range(BL):
                    for nhalf in range(2):
                        ps = pout.tile([128, 512], F32, tag="po")
                        for ko in range(NKO):
                            nc.tensor.matmul(
                                ps[:],
                                yt_bf[:, ko, :, b],
                                wo_sb[:, ko, nhalf * 512:(nhalf + 1) * 512],
                                start=(ko == 0), stop=(ko == NKO - 1),
                            )
                        osb = opool.tile([128, 512], F32, tag="osb")
                        nc.vector.tensor_tensor(
                            osb[:], ps[:], bob_sb[:, nhalf * 512:(nhalf + 1) * 512],
                            ALU.add,
                        )
                        nc.sync.dma_start(out[:, b, nhalf * 512:(nhalf + 1) * 512], osb[:])

    nc.finalize()
    return nc


_CACHED = {}


def _get_graph():
    if "nc" not in _CACHED:
        _CACHED["nc"] = build_graph()
    return _CACHED["nc"]


def make_in_maps(query, k_cache, v_cache, Wq, bq, Wk, bk, Wv, bv, Wo, bo):
    scale = np.float32(SCALE)
    wq_t = _tile_w(Wq.astype(np.float32) * scale)
    wk_t = _tile_w(Wk)
    wv_t = _tile_w(Wv)
    wo_t = _tile_w(Wo)
    bq_t = (bq.astype(np.float32) * scale).reshape(NKO, 128).T.copy()
    bk_t = bk.astype(np.float32).reshape(NKO, 128).T.copy()
    bvb_t = np.broadcast_to(bv.astype(np.float32), (128, E)).copy()
    bob_t = np.broadcast_to(bo.astype(np.float32), (128, E)).copy()

    in_maps = []
    for c in range(NCORES):
        bsl = slice(BL * c, BL * (c + 1))
        # xt[p, ko, s, b] = query[s, b, ko*128+p]
        xq = query[:, bsl, :].astype(bf16np)           # [S, BL, E]
        xt_t = np.ascontiguousarray(
            xq.reshape(S, BL, NKO, 128).transpose(3, 2, 0, 1)
        )
        # kt[b, hp, p, t] = k_cache[b, 2hp + p//64, t, p%64]
        kc = k_cache[bsl].astype(bf16np)               # [BL, NH, L, HD]
        kt_t = np.ascontiguousarray(
            kc.reshape(BL, NH // 2, 2, L, HD).transpose(0, 1, 2, 4, 3)
        ).reshape(BL, NH // 2, 128, L)
        # vt[b, h, p, tc, hd] = v_cache[b, h, tc*128+p, hd]
        vc = v_cache[bsl].astype(bf16np)               # [BL, NH, L, HD]
        vt_t = np.ascontiguousarray(
            vc.reshape(BL, NH, NTC, 128, HD).transpose(0, 1, 3, 2, 4)
        )
        in_maps.append({
            "xt": xt_t, "kt": kt_t, "vt": vt_t,
            "wq": wq_t, "wk": wk_t, "wv": wv_t, "wo": wo_t,
            "bq": bq_t, "bk": bk_t, "bvb": bvb_t, "bob": bob_t,
        })
    return in_maps


def _tile_w(W):
    # WT[ei, eo] tiled to [p, ki, eo]: row ei = ki*128+p
    WT = W.astype(np.float32).T.astype(bf16np)         # [E(in), E(out)]
    return np.ascontiguousarray(WT.reshape(NKO, 128, E).transpose(1, 0, 2))


def assemble_outputs(results, query, k_cache, v_cache):
    out = np.empty((S, B, E), np.float32)
    new_k = np.empty((B, NH, T, HD), np.float32)
    new_v = np.empty((B, NH, T, HD), np.float32)
    new_k[:, :, :L, :] = k_cache
    new_v[:, :, :L, :] = v_cache
    for c in range(NCORES):
        r = results[c]
        bsl = slice(BL * c, BL * (c + 1))
        out[:, bsl, :] = r["out"].reshape(128, BL, E)
        # knewt[p, ko, s, b] -> k_new[s, b, eo=ko*128+p] -> [b, n, s, hd]
        knt = r["knewt"].reshape(128, NKO, S, BL)
        k_new = knt.transpose(3, 1, 0, 2).reshape(BL, NKO * 128, S)  # [b, eo, s]
        k_new = k_new.reshape(BL, NH, HD, S).transpose(0, 1, 3, 2)   # [b, n, s, hd]
        new_k[bsl, :, L:, :] = k_new
        # vnew[b, s, eo] -> [b, n, s, hd]
        vn = r["vnew"].reshape(BL, S, NH, HD).transpose(0, 2, 1, 3)
        new_v[bsl, :, L:, :] = vn
    return out, new_k, new_v


def run_cores(in_maps, trace=False, **kwargs):
    nc = _get_graph()
    return run_bass_kernel_spmd(
        nc, in_maps, core_ids=list(range(NCORES)), trace=trace, **kwargs
    )


def kernel(query, key, k_cache, v_cache, Wq, bq, Wk, bk, Wv, bv, Wo, bo):
    in_maps = make_in_maps(query, k_cache, v_cache, Wq, bq, Wk, bk, Wv, bv, Wo, bo)
    res = run_cores(in_maps, trace=False)
    return assemble_outputs(res.results, query, k_cache, v_cache)


# revision 7
# speedup vs baseline: 1.5239x; 1.5239x over previous
"""Trainium2 Bass kernel: decoder self-attention with KV cache (AttentionForONNX).

Data-parallel over batch: B=16 batches sharded 2-per-core across 8 NeuronCores.
Each core computes q/k/v projections, attention over its 2 batches (16 heads,
L=2048 cache + S=128 new positions), and the output projection. No collectives.

Layout strategy (all matmuls natural, zero on-device transposes):
  - scores computed TRANSPOSED [t, s]: lhsT = kT chunk [hd=64, t=128]
    (even head at partitions 0-63, odd at 64-127 -> concurrent row-tiled MMs),
    rhs = qT head slice [hd=64, s=128].
  - softmax denominator via ones-column appended to V (extra PSUM row),
    no max-subtraction (scores are bounded ~|4|, exp is safe).
  - AV: lhsT = v_aug chunk [t=128, 65], rhs = expT chunk [t=128, s=128];
    output [65, s] accumulates over 17 chunks; row 64 = denominator.
  - divide: reciprocal rows -> one K=1 PE broadcast matmul -> bulk DVE multiply.
  - out projection: lhsT = YT chunk (stationary), rhs = WoT.
"""

import numpy as np
import ml_dtypes

import concourse.bass as bass
import concourse.mybir as mybir
from concourse import bacc
from concourse.tile import TileContext
from concourse.bass_utils import run_bass_kernel_spmd

AF = mybir.ActivationFunctionType
ALU = mybir.AluOpType
BF16 = mybir.dt.bfloat16
F32 = mybir.dt.float32
bf16np = ml_dtypes.bfloat16

S, B, E = 128, 16, 1024
NH, HD, L = 16, 64, 2048
NCORES = 8
BL = B // NCORES          # 2 batches per core
R = S * BL                # 256 rows (s-major, b-minor)
T = L + S                 # 2176 total kv positions
NTC = L // 128            # 16 cache chunks (+1 new chunk)
NKO = E // 128            # 8 e-blocks
SCALE = HD ** -0.5


def build_graph():
    nc = bacc.Bacc(target_bir_lowering=False)

    xt = nc.declare_dram_parameter("xt", [128, NKO, S, BL], BF16, isOutput=False)
    kt = nc.declare_dram_parameter("kt", [BL, NH // 2, 128, L], BF16, isOutput=False)
    vt = nc.declare_dram_parameter("vt", [BL, NH, 128, NTC, HD], BF16, isOutput=False)
    wq = nc.declare_dram_parameter("wq", [128, NKO, E], BF16, isOutput=False)
    wk = nc.declare_dram_parameter("wk", [128, NKO, E], BF16, isOutput=False)
    wv = nc.declare_dram_parameter("wv", [128, NKO, E], BF16, isOutput=False)
    wo = nc.declare_dram_parameter("wo", [128, NKO, E], BF16, isOutput=False)
    bq = nc.declare_dram_parameter("bq", [128, NKO], F32, isOutput=False)
    bk = nc.declare_dram_parameter("bk", [128, NKO], F32, isOutput=False)
    bvb = nc.declare_dram_parameter("bvb", [128, E], F32, isOutput=False)
    bob = nc.declare_dram_parameter("bob", [128, E], F32, isOutput=False)

    out = nc.declare_dram_parameter("out", [128, BL, E], F32, isOutput=True)
    knewt = nc.declare_dram_parameter("knewt", [128, NKO, S, BL], F32, isOutput=True)
    vnew = nc.declare_dram_parameter("vnew", [BL, 128, E], F32, isOutput=True)

    with TileContext(nc) as tc:
        with (
            tc.tile_pool(name="const", bufs=1) as cpool,
            tc.tile_pool(name="wpool", bufs=1) as wpool,
            tc.tile_pool(name="acts", bufs=1) as apool,
        ):
            # ---- persistent loads ----
            xt_sb = apool.tile([128, NKO, S, BL], BF16)
            nc.sync.dma_start(xt_sb[:], xt[:])
            wq_sb = wpool.tile([128, NKO, E], BF16, tag="wq")
            nc.sync.dma_start(wq_sb[:], wq[:])
            wk_sb = wpool.tile([128, NKO, E], BF16, tag="wk")
            nc.sync.dma_start(wk_sb[:], wk[:])
            wv_sb = wpool.tile([128, NKO, E], BF16, tag="wv")
            nc.sync.dma_start(wv_sb[:], wv[:])
            wo_sb = wpool.tile([128, NKO, E], BF16, tag="wo")
            nc.sync.dma_start(wo_sb[:], wo[:])
            bq_sb = cpool.tile([128, NKO], F32, tag="bq")
            nc.sync.dma_start(bq_sb[:], bq[:])
            bk_sb = cpool.tile([128, NKO], F32, tag="bk")
            nc.sync.dma_start(bk_sb[:], bk[:])
            bvb_sb = cpool.tile([128, E], F32, tag="bvb")
            nc.sync.dma_start(bvb_sb[:], bvb[:])
            bob_sb = cpool.tile([128, E], F32, tag="bob")
            nc.sync.dma_start(bob_sb[:], bob[:])
            ones_col = cpool.tile([1, 128], F32, tag="ones")
            nc.vector.memset(ones_col[:], 1.0)

            qt_bf = apool.tile([128, NKO, S, BL], BF16)
            knew_bf = apool.tile([128, NKO, S, BL], BF16)
            knew_f32 = apool.tile([128, NKO, S, BL], F32)
            vnew_f32 = [apool.tile([128, E], F32, tag=f"vnf{b}", name=f"vnf{b}") for b in range(BL)]
            vnew_aug = [apool.tile([128, NH, HD + 1], BF16, tag=f"vna{b}", name=f"vna{b}") for b in range(BL)]
            yt_f32 = apool.tile([128, NKO, S, BL], F32)
            yt_bf = apool.tile([128, NKO, S, BL], BF16)
            # reciprocal denominators, laid out [ko, half, s, b]
            drec = apool.tile([1, NKO, 2, S, BL], F32)

            # ---- phase 1: projections ----
            with tc.tile_pool(name="pproj", bufs=2, space="PSUM") as pp:
                for mo in range(NKO):
                    ps = pp.tile([128, R], F32, tag="pqk")
                    for ki in range(NKO):
                        nc.tensor.matmul(
                            ps[:],
                            wq_sb[:, ki, mo * 128:(mo + 1) * 128],
                            xt_sb[:, ki].rearrange("p s b -> p (s b)"),
                            start=(ki == 0), stop=(ki == NKO - 1),
                        )
                    nc.scalar.activation(
                        qt_bf[:, mo].rearrange("p s b -> p (s b)"), ps[:], AF.Identity,
                        bias=bq_sb[:, mo:mo + 1], scale=1.0,
                    )
                for mo in range(NKO):
                    ps = pp.tile([128, R], F32, tag="pqk")
                    for ki in range(NKO):
                        nc.tensor.matmul(
                            ps[:],
                            wk_sb[:, ki, mo * 128:(mo + 1) * 128],
                            xt_sb[:, ki].rearrange("p s b -> p (s b)"),
                            start=(ki == 0), stop=(ki == NKO - 1),
                        )
                    nc.scalar.activation(
                        knew_bf[:, mo].rearrange("p s b -> p (s b)"), ps[:], AF.Identity,
                        bias=bk_sb[:, mo:mo + 1], scale=1.0,
                    )
                    nc.vector.tensor_scalar_add(
                        knew_f32[:, mo].rearrange("p s b -> p (s b)"), ps[:], bk_sb[:, mo:mo + 1]
                    )
                nc.sync.dma_start(knewt[:], knew_f32[:])

                # v_new natural per batch: out[s, eo]
                for b in range(BL):
                    for nhalf in range(2):
                        ps = pp.tile([128, 512], F32, tag="pv")
                        for ki in range(NKO):
                            nc.tensor.matmul(
                                ps[:],
                                xt_sb[:, ki, :, b],
                                wv_sb[:, ki, nhalf * 512:(nhalf + 1) * 512],
                                start=(ki == 0), stop=(ki == NKO - 1),
                            )
                        nc.vector.tensor_tensor(
                            vnew_f32[b][:, nhalf * 512:(nhalf + 1) * 512],
                            ps[:], bvb_sb[:, nhalf * 512:(nhalf + 1) * 512],
                            ALU.add,
                        )
                    nc.sync.dma_start(vnew[b], vnew_f32[b][:])
                    nc.vector.tensor_copy(
                        out=vnew_aug[b][:, :, 0:HD],
                        in_=vnew_f32[b][:].rearrange("p (h d) -> p h d", d=HD),
                    )
                    nc.vector.memset(vnew_aug[b][:, :, HD:HD + 1], 1.0)

            # ---- phase 2: attention ----
            # Per head: 17 scoresT chunk matmuls into two multi-bank PSUM
            # tiles (8 + 9 chunks), ONE batched exp per tile (ACT cost is
            # (N+352)/1.2ns -- big N amortizes the fixed cost), then 17 AV
            # matmuls accumulating [65, s] (row 64 = softmax denominator).
            # AV for head h is emitted AFTER scores for head h+1 so the PE
            # never stalls waiting for ACT's exp.
            dstage = apool.tile([1, NKO, 2, S, BL], F32)
            with (
                tc.tile_pool(name="kp", bufs=3) as kpool,
                tc.tile_pool(name="vp", bufs=3) as vpool,
                tc.tile_pool(name="ep", bufs=2) as epool,
                tc.tile_pool(name="pssA", bufs=2, space="PSUM") as pssA,
                tc.tile_pool(name="pssB", bufs=1, space="PSUM") as pssB,
                tc.tile_pool(name="pso", bufs=1, space="PSUM") as pso,
            ):
                prev = None

                def do_av(pv):
                    b, hp, j, et, po = pv
                    h = 2 * hp + j
                    for tci in range(NTC + 1):
                        if tci < NTC:
                            lhsv = pv[5][:, tci, :]
                        else:
                            lhsv = vnew_aug[b][:, h, :]
                        nc.tensor.matmul(
                            po[:], lhsv, et[:, tci, :],
                            start=(tci == 0), stop=(tci == NTC),
                        )
                    nc.vector.tensor_copy(
                        out=dstage[0:1, hp, j, :, b], in_=po[64:65, :]
                    )
                    nc.vector.tensor_copy(
                        out=yt_f32[64 * j:64 * (j + 1), hp, :, b],
                        in_=po[0:64, :],
                    )

                for b in range(BL):
                    for hp in range(NH // 2):
                        kslab = kpool.tile([128, L], BF16, tag="kslab")
                        nc.sync.dma_start(kslab[:], kt[b, hp])
                        vaug = []
                        for j in range(2):
                            va = vpool.tile([128, NTC, HD + 1], BF16, tag=f"vaug{j}", name=f"vaug{j}")
                            nc.sync.dma_start(va[:, :, 0:HD], vt[b, 2 * hp + j])
                            nc.vector.memset(va[:, :, HD:HD + 1], 1.0)
                            vaug.append(va)

                        for j in range(2):
                            qh = qt_bf[64 * j:64 * (j + 1), hp, :, b]
                            tA = pssA.tile([128, 8, 128], F32, tag="sA")
                            tB = pssB.tile([128, 9, 128], F32, tag="sB")
                            po = pso.tile([65, 128], F32, tag="po")
                            et = epool.tile([128, NTC + 1, 128], BF16, tag="exp")
                            for tci in range(8):
                                lhs = kslab[64 * j:64 * (j + 1), tci * 128:(tci + 1) * 128]
                                nc.tensor.matmul(tA[:, tci, :], lhs, qh, start=True, stop=True)
                            nc.scalar.activation(et[:, 0:8, :], tA[:], AF.Exp)
                            for tci in range(8, NTC + 1):
                                if tci < NTC:
                                    lhs = kslab[64 * j:64 * (j + 1), tci * 128:(tci + 1) * 128]
                                else:
                                    lhs = knew_bf[64 * j:64 * (j + 1), hp, :, b]
                                nc.tensor.matmul(tB[:, tci - 8, :], lhs, qh, start=True, stop=True)
                            nc.scalar.activation(et[:, 8:NTC + 1, :], tB[:], AF.Exp)
                            if prev is not None:
                                do_av(prev)
                            prev = (b, hp, j, et, po, vaug[j])
                do_av(prev)
            # one batched reciprocal over all 4096 denominators
            nc.vector.reciprocal(
                drec[:].rearrange("p a b s c -> p (a b s c)"),
                dstage[:].rearrange("p a b s c -> p (a b s c)"),
            )

            # ---- phase 3: divide + out projection ----
            with (
                tc.tile_pool(name="pbc", bufs=2, space="PSUM") as pbc,
                tc.tile_pool(name="pout", bufs=2, space="PSUM") as pout,
                tc.tile_pool(name="osb", bufs=3) as opool,
            ):
                for ko in range(NKO):
                    rb = pbc.tile([128, 512], F32, tag="rb")
                    nc.tensor.matmul(
                        rb[:], ones_col[:], drec[0:1, ko].rearrange("p a s b -> p (a s b)"),
                        start=True, stop=True,
                    )
                    for j in range(2):
                        nc.vector.tensor_tensor(
                            yt_bf[64 * j:64 * (j + 1), ko, :, :],
                            yt_f32[64 * j:64 * (j + 1), ko, :, :],
                            rb[64 * j:64 * (j + 1), j * 256:(j + 1) * 256].rearrange("p (s b) -> p s b", b=BL),
                            ALU.mult,
                        )

                for b in range(BL):
                    for nhalf in range(2):
                        ps = pout.tile([128, 512], F32, tag="po")
                        for ko in range(NKO):
                            nc.tensor.matmul(
                                ps[:],
                                yt_bf[:, ko, :, b],
                                wo_sb[:, ko, nhalf * 512:(nhalf + 1) * 512],
                                start=(ko == 0), stop=(ko == NKO - 1),
                            )
                        osb = opool.tile([128, 512], F32, tag="osb")
                        nc.vector.tensor_tensor(
                            osb[:], ps[:], bob_sb[:, nhalf * 512:(nhalf + 1) * 512],
                            ALU.add,
                        )
                        nc.sync.dma_start(out[:, b, nhalf * 512:(nhalf + 1) * 512], osb[:])

    nc.finalize()
    return nc


_CACHED = {}


def _get_graph():
    if "nc" not in _CACHED:
        _CACHED["nc"] = build_graph()
    return _CACHED["nc"]


def make_in_maps(query, k_cache, v_cache, Wq, bq, Wk, bk, Wv, bv, Wo, bo):
    scale = np.float32(SCALE)
    wq_t = _tile_w(Wq.astype(np.float32) * scale)
    wk_t = _tile_w(Wk)
    wv_t = _tile_w(Wv)
    wo_t = _tile_w(Wo)
    bq_t = (bq.astype(np.float32) * scale).reshape(NKO, 128).T.copy()
    bk_t = bk.astype(np.float32).reshape(NKO, 128).T.copy()
    bvb_t = np.broadcast_to(bv.astype(np.float32), (128, E)).copy()
    bob_t = np.broadcast_to(bo.astype(np.float32), (128, E)).copy()

    in_maps = []
    for c in range(NCORES):
        bsl = slice(BL * c, BL * (c + 1))
        # xt[p, ko, s, b] = query[s, b, ko*128+p]
        xq = query[:, bsl, :].astype(bf16np)           # [S, BL, E]
        xt_t = np.ascontiguousarray(
            xq.reshape(S, BL, NKO, 128).transpose(3, 2, 0, 1)
        )
        # kt[b, hp, p, t] = k_cache[b, 2hp + p//64, t, p%64]
        kc = k_cache[bsl].astype(bf16np)               # [BL, NH, L, HD]
        kt_t = np.ascontiguousarray(
            kc.reshape(BL, NH // 2, 2, L, HD).transpose(0, 1, 2, 4, 3)
        ).reshape(BL, NH // 2, 128, L)
        # vt[b, h, p, tc, hd] = v_cache[b, h, tc*128+p, hd]
        vc = v_cache[bsl].astype(bf16np)               # [BL, NH, L, HD]
        vt_t = np.ascontiguousarray(
            vc.reshape(BL, NH, NTC, 128, HD).transpose(0, 1, 3, 2, 4)
        )
        in_maps.append({
            "xt": xt_t, "kt": kt_t, "vt": vt_t,
            "wq": wq_t, "wk": wk_t, "wv": wv_t, "wo": wo_t,
            "bq": bq_t, "bk": bk_t, "bvb": bvb_t, "bob": bob_t,
        })
    return in_maps


def _tile_w(W):
    # WT[ei, eo] tiled to [p, ki, eo]: row ei = ki*128+p
    WT = W.astype(np.float32).T.astype(bf16np)         # [E(in), E(out)]
    return np.ascontiguousarray(WT.reshape(NKO, 128, E).transpose(1, 0, 2))


def assemble_outputs(results, query, k_cache, v_cache):
    out = np.empty((S, B, E), np.float32)
    new_k = np.empty((B, NH, T, HD), np.float32)
    new_v = np.empty((B, NH, T, HD), np.float32)
    new_k[:, :, :L, :] = k_cache
    new_v[:, :, :L, :] = v_cache
    for c in range(NCORES):
        r = results[c]
        bsl = slice(BL * c, BL * (c + 1))
        out[:, bsl, :] = r["out"].reshape(128, BL, E)
        # knewt[p, ko, s, b] -> k_new[s, b, eo=ko*128+p] -> [b, n, s, hd]
        knt = r["knewt"].reshape(128, NKO, S, BL)
        k_new = knt.transpose(3, 1, 0, 2).reshape(BL, NKO * 128, S)  # [b, eo, s]
        k_new = k_new.reshape(BL, NH, HD, S).transpose(0, 1, 3, 2)   # [b, n, s, hd]
        new_k[bsl, :, L:, :] = k_new
        # vnew[b, s, eo] -> [b, n, s, hd]
        vn = r["vnew"].reshape(BL, S, NH, HD).transpose(0, 2, 1, 3)
        new_v[bsl, :, L:, :] = vn
    return out, new_k, new_v


def run_cores(in_maps, trace=False, **kwargs):
    nc = _get_graph()
    return run_bass_kernel_spmd(
        nc, in_maps, core_ids=list(range(NCORES)), trace=trace, **kwargs
    )


def kernel(query, key, k_cache, v_cache, Wq, bq, Wk, bk, Wv, bv, Wo, bo):
    in_maps = make_in_maps(query, k_cache, v_cache, Wq, bq, Wk, bk, Wv, bv, Wo, bo)
    res = run_cores(in_maps, trace=False)
    return assemble_outputs(res.results, query, k_cache, v_cache)


# revision 10
# speedup vs baseline: 1.6669x; 1.0938x over previous
"""Trainium2 Bass kernel: decoder self-attention with KV cache (AttentionForONNX).

Data-parallel over batch: B=16 batches sharded 2-per-core across 8 NeuronCores.
Each core computes q/k/v projections, attention over its 2 batches (16 heads,
L=2048 cache + S=128 new positions), and the output projection. No collectives.

Layout strategy (all matmuls natural, zero on-device transposes):
  - scores computed TRANSPOSED [t, s]: lhsT = kT chunk [hd=64, t=128]
    (even head at partitions 0-63, odd at 64-127 -> concurrent row-tiled MMs),
    rhs = qT head slice [hd=64, s=128].
  - softmax denominator via ones-column appended to V (extra PSUM row),
    no max-subtraction (scores are bounded ~|4|, exp is safe).
  - AV: lhsT = v_aug chunk [t=128, 65], rhs = expT chunk [t=128, s=128];
    output [65, s] accumulates over 17 chunks; row 64 = denominator.
  - divide: reciprocal rows -> one K=1 PE broadcast matmul -> bulk DVE multiply.
  - out projection: lhsT = YT chunk (stationary), rhs = WoT.
"""

import numpy as np
import ml_dtypes

import concourse.bass as bass
import concourse.mybir as mybir
from concourse import bacc
from concourse.tile import TileContext
from concourse.bass_utils import run_bass_kernel_spmd

AF = mybir.ActivationFunctionType
ALU = mybir.AluOpType
BF16 = mybir.dt.bfloat16
F32 = mybir.dt.float32
bf16np = ml_dtypes.bfloat16

S, B, E = 128, 16, 1024
NH, HD, L = 16, 64, 2048
NCORES = 8
BL = B // NCORES          # 2 batches per core
R = S * BL                # 256 rows (s-major, b-minor)
T = L + S                 # 2176 total kv positions
NTC = L // 128            # 16 cache chunks (+1 new chunk)
NKO = E // 128            # 8 e-blocks
SCALE = HD ** -0.5


def build_graph():
    nc = bacc.Bacc(target_bir_lowering=False)

    xt = nc.declare_dram_parameter("xt", [128, NKO, S, BL], BF16, isOutput=False)
    kt = nc.declare_dram_parameter("kt", [BL, NH // 2, 128, L], BF16, isOutput=False)
    vt = nc.declare_dram_parameter("vt", [BL, NH, 128, NTC, HD], BF16, isOutput=False)
    wq = nc.declare_dram_parameter("wq", [128, NKO, E], BF16, isOutput=False)
    wk = nc.declare_dram_parameter("wk", [128, NKO, E], BF16, isOutput=False)
    wv = nc.declare_dram_parameter("wv", [128, NKO, E], BF16, isOutput=False)
    wo = nc.declare_dram_parameter("wo", [128, NKO, E], BF16, isOutput=False)
    bq = nc.declare_dram_parameter("bq", [128, NKO], F32, isOutput=False)
    bk = nc.declare_dram_parameter("bk", [128, NKO], F32, isOutput=False)
    bvb = nc.declare_dram_parameter("bvb", [128, E], F32, isOutput=False)
    bob = nc.declare_dram_parameter("bob", [128, E], F32, isOutput=False)

    out = nc.declare_dram_parameter("out", [128, BL, E], F32, isOutput=True)
    knewt = nc.declare_dram_parameter("knewt", [128, NKO, S, BL], F32, isOutput=True)
    vnew = nc.declare_dram_parameter("vnew", [BL, 128, E], F32, isOutput=True)

    with TileContext(nc) as tc:
        with (
            tc.tile_pool(name="const", bufs=1) as cpool,
            tc.tile_pool(name="wpool", bufs=1) as wpool,
            tc.tile_pool(name="acts", bufs=1) as apool,
        ):
            # ---- persistent loads ----
            xt_sb = apool.tile([128, NKO, S, BL], BF16)
            nc.sync.dma_start(xt_sb[:], xt[:])
            wq_sb = wpool.tile([128, NKO, E], BF16, tag="wq")
            wk_sb = wpool.tile([128, NKO, E], BF16, tag="wk")
            wv_sb = wpool.tile([128, NKO, E], BF16, tag="wv")
            wo_sb = wpool.tile([128, NKO, E], BF16, tag="wo")
            # per-ki slice DMAs so the first projection matmuls only wait on
            # their own 256KB slice; wo last (needed at the very end)
            for ki in range(NKO):
                nc.sync.dma_start(wq_sb[:, ki], wq[:, ki])
            bq_sb = cpool.tile([128, NKO], F32, tag="bq")
            nc.sync.dma_start(bq_sb[:], bq[:])
            for ki in range(NKO):
                nc.sync.dma_start(wk_sb[:, ki], wk[:, ki])
            bk_sb = cpool.tile([128, NKO], F32, tag="bk")
            nc.sync.dma_start(bk_sb[:], bk[:])
            for ki in range(NKO):
                nc.sync.dma_start(wv_sb[:, ki], wv[:, ki])
            bvb_sb = cpool.tile([128, E], F32, tag="bvb")
            nc.sync.dma_start(bvb_sb[:], bvb[:])
            for ki in range(NKO):
                nc.sync.dma_start(wo_sb[:, ki], wo[:, ki])
            bob_sb = cpool.tile([128, E], F32, tag="bob")
            nc.sync.dma_start(bob_sb[:], bob[:])
            ones_col = cpool.tile([1, 128], F32, tag="ones")
            nc.vector.memset(ones_col[:], 1.0)

            qt_bf = apool.tile([128, NKO, S, BL], BF16)
            knew_bf = apool.tile([128, NKO, S, BL], BF16)
            knew_f32 = apool.tile([128, NKO, S, BL], F32)
            vnew_f32 = [apool.tile([128, E], F32, tag=f"vnf{b}", name=f"vnf{b}") for b in range(BL)]
            vnew_aug = [apool.tile([128, NH, HD + 1], BF16, tag=f"vna{b}", name=f"vna{b}") for b in range(BL)]
            yt_f32 = apool.tile([128, NKO, S, BL], F32)
            yt_bf = apool.tile([128, NKO, S, BL], BF16)
            # reciprocal denominators, laid out [ko, half, s, b]
            drec = apool.tile([1, NKO, 2, S, BL], F32)

            # ---- phase 1: projections ----
            with tc.tile_pool(name="pproj", bufs=2, space="PSUM") as pp:
                for mo in range(NKO):
                    ps = pp.tile([128, R], F32, tag="pqk")
                    for ki in range(NKO):
                        nc.tensor.matmul(
                            ps[:],
                            wq_sb[:, ki, mo * 128:(mo + 1) * 128],
                            xt_sb[:, ki].rearrange("p s b -> p (s b)"),
                            start=(ki == 0), stop=(ki == NKO - 1),
                        )
                    nc.scalar.activation(
                        qt_bf[:, mo].rearrange("p s b -> p (s b)"), ps[:], AF.Identity,
                        bias=bq_sb[:, mo:mo + 1], scale=1.0,
                    )
                for mo in range(NKO):
                    ps = pp.tile([128, R], F32, tag="pqk")
                    for ki in range(NKO):
                        nc.tensor.matmul(
                            ps[:],
                            wk_sb[:, ki, mo * 128:(mo + 1) * 128],
                            xt_sb[:, ki].rearrange("p s b -> p (s b)"),
                            start=(ki == 0), stop=(ki == NKO - 1),
                        )
                    nc.scalar.activation(
                        knew_bf[:, mo].rearrange("p s b -> p (s b)"), ps[:], AF.Identity,
                        bias=bk_sb[:, mo:mo + 1], scale=1.0,
                    )
                    nc.vector.tensor_scalar_add(
                        knew_f32[:, mo].rearrange("p s b -> p (s b)"), ps[:], bk_sb[:, mo:mo + 1]
                    )
                nc.sync.dma_start(knewt[:], knew_f32[:])

                # v_new natural per batch: out[s, eo]
                for b in range(BL):
                    for nhalf in range(2):
                        ps = pp.tile([128, 512], F32, tag="pv")
                        for ki in range(NKO):
                            nc.tensor.matmul(
                                ps[:],
                                xt_sb[:, ki, :, b],
                                wv_sb[:, ki, nhalf * 512:(nhalf + 1) * 512],
                                start=(ki == 0), stop=(ki == NKO - 1),
                            )
                        nc.vector.tensor_tensor(
                            vnew_f32[b][:, nhalf * 512:(nhalf + 1) * 512],
                            ps[:], bvb_sb[:, nhalf * 512:(nhalf + 1) * 512],
                            ALU.add,
                        )
                    nc.sync.dma_start(vnew[b], vnew_f32[b][:])
                    nc.vector.tensor_copy(
                        out=vnew_aug[b][:, :, 0:HD],
                        in_=vnew_f32[b][:].rearrange("p (h d) -> p h d", d=HD),
                    )
                    nc.vector.memset(vnew_aug[b][:, :, HD:HD + 1], 1.0)

            # ---- phase 2: attention (+ per-batch divide & out-projection) ----
            # Per head: 17 scoresT chunk matmuls into two multi-bank PSUM
            # tiles (8 + 9 chunks), ONE batched exp per tile (ACT cost is
            # (N+352)/1.2ns -- big N amortizes the fixed cost), then 17 AV
            # matmuls accumulating [65, s] (row 64 = softmax denominator).
            # AV for head h is emitted AFTER scores for head h+1 so the PE
            # never stalls waiting for ACT's exp. After each batch's 16
            # heads, its softmax divide + out-projection are emitted
            # (PSUM slots borrowed from the attention pools), overlapping
            # the other batch's attention.
            dstage = apool.tile([1, NKO, 2, S, BL], F32)
            F32R = mybir.dt.float32r
            with (
                tc.tile_pool(name="kp", bufs=3) as kpool,
                tc.tile_pool(name="vp", bufs=3) as vpool,
                tc.tile_pool(name="ep", bufs=2) as epool,
                tc.tile_pool(name="osb", bufs=3) as opool,
                tc.tile_pool(name="pssA", bufs=2, space="PSUM") as pssA,
                tc.tile_pool(name="pssB", bufs=1, space="PSUM") as pssB,
                tc.tile_pool(name="pso", bufs=1, space="PSUM") as pso,
            ):
                prev = None

                def do_av(pv):
                    b, hp, j, et, po, va = pv
                    h = 2 * hp + j
                    for tci in range(NTC + 1):
                        if tci < NTC:
                            lhsv = va[:, tci, :]
                        else:
                            lhsv = vnew_aug[b][:, h, :]
                        nc.tensor.matmul(
                            po[:], lhsv, et[:, tci, :],
                            start=(tci == 0), stop=(tci == NTC),
                        )
                    nc.vector.tensor_copy(
                        out=dstage[0:1, hp, j, :, b], in_=po[64:65, :]
                    )
                    nc.vector.tensor_copy(
                        out=yt_f32[64 * j:64 * (j + 1), hp, :, b],
                        in_=po[0:64, :],
                    )

                def do_output(b):
                    # reciprocal of this batch's 2048 denominators in one op
                    nc.vector.reciprocal(
                        drec[0:1, :, :, :, b].rearrange("p a c s -> p (a c s)"),
                        dstage[0:1, :, :, :, b].rearrange("p a c s -> p (a c s)"),
                    )
                    for ko in range(NKO):
                        rb = pssA.tile([128, 256], F32, tag="sA", name="rb")
                        nc.tensor.matmul(
                            rb[:],
                            ones_col[:].bitcast(F32R),
                            drec[0:1, ko, :, :, b].rearrange("p c s -> p (c s)").bitcast(F32R),
                            start=True, stop=True,
                        )
                        for j in range(2):
                            nc.vector.tensor_tensor(
                                yt_bf[64 * j:64 * (j + 1), ko, :, b],
                                yt_f32[64 * j:64 * (j + 1), ko, :, b],
                                rb[64 * j:64 * (j + 1), j * 128:(j + 1) * 128],
                                ALU.mult,
                            )
                    for nhalf in range(2):
                        ps = pssB.tile([128, 512], F32, tag="sB", name="psout")
                        for ko in range(NKO):
                            nc.tensor.matmul(
                                ps[:],
                                yt_bf[:, ko, :, b],
                                wo_sb[:, ko, nhalf * 512:(nhalf + 1) * 512],
                                start=(ko == 0), stop=(ko == NKO - 1),
                            )
                        osb = opool.tile([128, 512], F32, tag="osb", name="osb")
                        nc.vector.tensor_tensor(
                            osb[:], ps[:], bob_sb[:, nhalf * 512:(nhalf + 1) * 512],
                            ALU.add,
                        )
                        nc.sync.dma_start(out[:, b, nhalf * 512:(nhalf + 1) * 512], osb[:])

                for b in range(BL):
                    for hp in range(NH // 2):
                        kslab = kpool.tile([128, L], BF16, tag="kslab")
                        nc.gpsimd.dma_start(kslab[:], kt[b, hp])
                        vaug = []
                        for j in range(2):
                            va = vpool.tile([128, NTC, HD + 1], BF16, tag=f"vaug{j}", name=f"vaug{j}")
                            nc.gpsimd.dma_start(va[:, :, 0:HD], vt[b, 2 * hp + j])
                            nc.vector.memset(va[:, :, HD:HD + 1], 1.0)
                            vaug.append(va)

                        for j in range(2):
                            qh = qt_bf[64 * j:64 * (j + 1), hp, :, b]
                            tA = pssA.tile([128, 8, 128], F32, tag="sA")
                            tB = pssB.tile([128, 9, 128], F32, tag="sB")
                            po = pso.tile([65, 128], F32, tag="po")
                            et = epool.tile([128, NTC + 1, 128], BF16, tag="exp")
                            for tci in range(8):
                                lhs = kslab[64 * j:64 * (j + 1), tci * 128:(tci + 1) * 128]
                                nc.tensor.matmul(tA[:, tci, :], lhs, qh, start=True, stop=True)
                            nc.scalar.activation(et[:, 0:8, :], tA[:], AF.Exp)
                            for tci in range(8, NTC + 1):
                                if tci < NTC:
                                    lhs = kslab[64 * j:64 * (j + 1), tci * 128:(tci + 1) * 128]
                                else:
                                    lhs = knew_bf[64 * j:64 * (j + 1), hp, :, b]
                                nc.tensor.matmul(tB[:, tci - 8, :], lhs, qh, start=True, stop=True)
                            nc.scalar.activation(et[:, 8:NTC + 1, :], tB[:], AF.Exp)
                            if prev is not None:
                                do_av(prev)
                            prev = (b, hp, j, et, po, vaug[j])
                    # flush the last head of this batch, then emit its
                    # divide + out-projection (overlaps next batch's attention)
                    do_av(prev)
                    prev = None
                    do_output(b)

    nc.finalize()
    return nc


_CACHED = {}


def _get_graph():
    if "nc" not in _CACHED:
        _CACHED["nc"] = build_graph()
    return _CACHED["nc"]


def make_in_maps(query, k_cache, v_cache, Wq, bq, Wk, bk, Wv, bv, Wo, bo):
    scale = np.float32(SCALE)
    wq_t = _tile_w(Wq.astype(np.float32) * scale)
    wk_t = _tile_w(Wk)
    wv_t = _tile_w(Wv)
    wo_t = _tile_w(Wo)
    bq_t = (bq.astype(np.float32) * scale).reshape(NKO, 128).T.copy()
    bk_t = bk.astype(np.float32).reshape(NKO, 128).T.copy()
    bvb_t = np.broadcast_to(bv.astype(np.float32), (128, E)).copy()
    bob_t = np.broadcast_to(bo.astype(np.float32), (128, E)).copy()

    in_maps = []
    for c in range(NCORES):
        bsl = slice(BL * c, BL * (c + 1))
        # xt[p, ko, s, b] = query[s, b, ko*128+p]
        xq = query[:, bsl, :].astype(bf16np)           # [S, BL, E]
        xt_t = np.ascontiguousarray(
            xq.reshape(S, BL, NKO, 128).transpose(3, 2, 0, 1)
        )
        # kt[b, hp, p, t] = k_cache[b, 2hp + p//64, t, p%64]
        kc = k_cache[bsl].astype(bf16np)               # [BL, NH, L, HD]
        kt_t = np.ascontiguousarray(
            kc.reshape(BL, NH // 2, 2, L, HD).transpose(0, 1, 2, 4, 3)
        ).reshape(BL, NH // 2, 128, L)
        # vt[b, h, p, tc, hd] = v_cache[b, h, tc*128+p, hd]
        vc = v_cache[bsl].astype(bf16np)               # [BL, NH, L, HD]
        vt_t = np.ascontiguousarray(
            vc.reshape(BL, NH, NTC, 128, HD).transpose(0, 1, 3, 2, 4)
        )
        in_maps.append({
            "xt": xt_t, "kt": kt_t, "vt": vt_t,
            "wq": wq_t, "wk": wk_t, "wv": wv_t, "wo": wo_t,
            "bq": bq_t, "bk": bk_t, "bvb": bvb_t, "bob": bob_t,
        })
    return in_maps


def _tile_w(W):
    # WT[ei, eo] tiled to [p, ki, eo]: row ei = ki*128+p
    WT = W.astype(np.float32).T.astype(bf16np)         # [E(in), E(out)]
    return np.ascontiguousarray(WT.reshape(NKO, 128, E).transpose(1, 0, 2))


def assemble_outputs(results, query, k_cache, v_cache):
    out = np.empty((S, B, E), np.float32)
    new_k = np.empty((B, NH, T, HD), np.float32)
    new_v = np.empty((B, NH, T, HD), np.float32)
    new_k[:, :, :L, :] = k_cache
    new_v[:, :, :L, :] = v_cache
    for c in range(NCORES):
        r = results[c]
        bsl = slice(BL * c, BL * (c + 1))
        out[:, bsl, :] = r["out"].reshape(128, BL, E)
        # knewt[p, ko, s, b] -> k_new[s, b, eo=ko*128+p] -> [b, n, s, hd]
        knt = r["knewt"].reshape(128, NKO, S, BL)
        k_new = knt.transpose(3, 1, 0, 2).reshape(BL, NKO * 128, S)  # [b, eo, s]
        k_new = k_new.reshape(BL, NH, HD, S).transpose(0, 1, 3, 2)   # [b, n, s, hd]
        new_k[bsl, :, L:, :] = k_new
        # vnew[b, s, eo] -> [b, n, s, hd]
        vn = r["vnew"].reshape(BL, S, NH, HD).transpose(0, 2, 1, 3)
        new_v[bsl, :, L:, :] = vn
    return out, new_k, new_v


def run_cores(in_maps, trace=False, **kwargs):
    nc = _get_graph()
    return run_bass_kernel_spmd(
        nc, in_maps, core_ids=list(range(NCORES)), trace=trace, **kwargs
    )


def kernel(query, key, k_cache, v_cache, Wq, bq, Wk, bk, Wv, bv, Wo, bo):
    in_maps = make_in_maps(query, k_cache, v_cache, Wq, bq, Wk, bk, Wv, bv, Wo, bo)
    res = run_cores(in_maps, trace=False)
    return assemble_outputs(res.results, query, k_cache, v_cache)
